# revision 21
# baseline (speedup 1.0000x reference)
"""Trainium2 Bass kernel for nn_DiffConvAdaptive (B=32, N=1024, C=768, K=3).

Sharding: data-parallel over batch, 8 cores x 4 samples, no collectives.

v2 architecture — the depthwise 3x3 conv is split across PE/ACT/DVE
(measured op costs; GPSIMD elementwise poisons DVE via the shared SBUF
port, so it only does memsets):
  PE:  pooling (S.T @ x), kernel-gen, p2, proj, and 3 conv taps as
       diagonal-stationary matmuls accumulating in PSUM.
  ACT: p2 PSUM->workspace evacuation (+bias, ->bf16), conv center-tap
       written straight from p2 PSUM with per-channel scale (the
       accumulator init), 2 tap products, PE-partial evacuation.
  DVE: 3 fused scalar_tensor_tensor taps + 3 tensor_tensor folds
       (2 ACT products + 1 PE partial), diag-matrix builds.
Chunks are software-pipelined (stage i emits p2(i), taps(i),
PE-conv(i-1), folds(i-1)) so no engine queue head-of-line blocks.

Kernel generation (adaptive pool + p1 + silu + kernel_gen) is batched
across the 4 local samples: pooling commutes with p1 (pool raw x with
the [1024, 9] segment matrix); the "k - sigmoid(beta)*mean(k)"
correction is folded into kg_w/kg_b on the host (sigmoid(beta)
uniform). proj bias is added on the host after the gather.
"""

import sys

if "/opt/trn_rl_repo" not in sys.path:
    sys.path.insert(0, "/opt/trn_rl_repo")

import numpy as np
import ml_dtypes

import concourse.bass as bass
import concourse.bacc as bacc
import concourse.mybir as mybir
import concourse.tile as tile
from concourse.bass_utils import run_bass_kernel_spmd

N_CORES = 8
B, N, C = 32, 1024, 768
B_LOC = B // N_CORES
KK = 9
NCH = C // 128   # 6 channel chunks
NTC = N // 128   # 8 token chunks

# padded conv workspace: 34 rows x 36 cols (row stride 36), 32x32 image,
# interior pixel (y, x) at offset 36*(y+1) + 1 + x. Tap (dy, dx) reads a
# [32 rows stride 36][32 cols] view at base 36*dy + dx (+576*h for the
# 16-row halves used by the PE taps).
WS = 36 * 34 + 4

BF = mybir.dt.bfloat16
F32 = mybir.dt.float32

# tap plan: 4 taps on the PE (diag matmuls in PSUM), tap (0,0) is the
# DVE tensor_scalar accumulator init, 2 more fused DVE taps, 2 ACT
# products (folded by DVE).
PE_TAPS = [(0, 1), (2, 1), (1, 0)]
TS_INIT = (0, 0)
STT_TAPS = [(0, 2), (2, 0)]
ACT_TAPS = [(1, 2), (2, 2), (1, 1)]
# kT column order (permuted via the transpose identity so the PE taps
# occupy contiguous columns 0..3 for the broadcast dg build)
TAP_ORDER = PE_TAPS + [TS_INIT] + STT_TAPS + ACT_TAPS
TAP_COL = {t: j for j, t in enumerate(TAP_ORDER)}
NPE = len(PE_TAPS)

_CACHE = {}
LAST_RESULTS = None


def _segment_matrix():
    S = np.zeros((N, KK), np.float32)
    for i in range(KK):
        s = (i * N) // KK
        e = -((-(i + 1) * N) // KK)
        S[s:e, i] = 1.0 / (e - s)
    return S


def build_program():
    nc = bacc.Bacc(None)

    x_d = nc.dram_tensor("xbf", [B_LOC, N, C], BF, kind="ExternalInput")
    wp2T_d = nc.dram_tensor("wp2T", [128, NCH * C], BF, kind="ExternalInput")
    wp1T_d = nc.dram_tensor("wp1T", [128, NCH * C], BF, kind="ExternalInput")
    wprojT_d = nc.dram_tensor("wprojT", [128, NCH * C], BF,
                              kind="ExternalInput")
    S_d = nc.dram_tensor("S", [128, NTC * KK], BF, kind="ExternalInput")
    kg4_d = nc.dram_tensor("kg4", [4 * KK, 4 * KK], BF, kind="ExternalInput")
    p1b_d = nc.dram_tensor("p1b", [1, C], BF, kind="ExternalInput")
    p2bT_d = nc.dram_tensor("p2bT", [128, NCH], F32, kind="ExternalInput")
    kgb4_d = nc.dram_tensor("kgb4", [4 * KK, 1], F32, kind="ExternalInput")
    ones_d = nc.dram_tensor("ones", [1, 4 * KK], BF, kind="ExternalInput")
    eye9b_d = nc.dram_tensor("eye9b", [KK, KK], BF, kind="ExternalInput")
    p36_d = nc.dram_tensor("p36", [4 * KK, 4 * KK], F32,
                           kind="ExternalInput")
    eye3x_d = nc.dram_tensor("eye3x", [128, NPE * 128], BF,
                             kind="ExternalInput")
    out_d = nc.dram_tensor("out", [B_LOC, N, C], F32, kind="ExternalOutput")

    add = mybir.AluOpType.add
    mult = mybir.AluOpType.mult
    IDENT = mybir.ActivationFunctionType.Identity

    with tile.TileContext(nc) as tc:
        with (
            tc.tile_pool(name="const", bufs=1) as cpool,
            tc.tile_pool(name="ws", bufs=1) as wspool,
            tc.tile_pool(name="x4", bufs=6) as x4pool,
            tc.tile_pool(name="xt", bufs=18) as xtpool,
            tc.tile_pool(name="cv", bufs=13) as cvpool,
            tc.tile_pool(name="tmp", bufs=8) as tmppool,
            tc.tile_pool(name="dg", bufs=4) as dgpool,
            tc.tile_pool(name="kgen", bufs=2) as kgpool,
            tc.tile_pool(name="ktp", bufs=12) as ktpool,
            tc.tile_pool(name="osb", bufs=4) as osbpool,
            tc.tile_pool(name="psA", bufs=4, space="PSUM") as psA,
            tc.tile_pool(name="psB", bufs=4, space="PSUM") as psB,
        ):
            # ---------------- DMA issues ----------------
            # HWDGE ring latency is ~0.7-1.3us PER DMA, FIFO — so the
            # small consts ride the otherwise-idle GPSIMD SWDGE queue,
            # keeping both HWDGE rings clear for x4 (which gates
            # pooling -> kgen -> every conv chunk).
            S_sb = cpool.tile([128, NTC * KK], BF, tag="S")
            nc.scalar.dma_start(S_sb[:], S_d[:])
            p2bT_sb = cpool.tile([128, NCH], F32, tag="p2bT")
            nc.gpsimd.dma_start(p2bT_sb[:], p2bT_d[:])
            kg4_sb = cpool.tile([4 * KK, 4 * KK], BF, tag="kg4")
            nc.gpsimd.dma_start(kg4_sb[:], kg4_d[:])
            p1b_sb = cpool.tile([1, C], BF, tag="p1b")
            nc.gpsimd.dma_start(p1b_sb[:], p1b_d[:])
            kgb4_sb = cpool.tile([4 * KK, 1], F32, tag="kgb4")
            nc.gpsimd.dma_start(kgb4_sb[:], kgb4_d[:])
            ones_sb = cpool.tile([1, 4 * KK], BF, tag="ones")
            nc.gpsimd.dma_start(ones_sb[:], ones_d[:])
            eye9b = cpool.tile([KK, KK], BF, tag="eye9b")
            nc.gpsimd.dma_start(eye9b[:], eye9b_d[:])
            p36_sb = cpool.tile([4 * KK, 4 * KK], F32, tag="p36")
            nc.gpsimd.dma_start(p36_sb[:], p36_d[:])
            eye3x = cpool.tile([128, NPE * 128], BF, tag="eye3x")
            nc.gpsimd.dma_start(eye3x[:], eye3x_d[:])
            # token-major x split across both hwdge queues (even on SP,
            # odd on ACT) so the wire fills from both sides
            x4 = []
            for t in range(NTC):
                x4t = x4pool.tile([128, B_LOC * C], BF, tag="x4")
                x4.append(x4t)
            for t in range(0, NTC, 2):
                nc.sync.dma_start(
                    x4[t][:].rearrange("p (b c) -> p b c", b=B_LOC),
                    x_d[:, 128 * t:128 * (t + 1), :].rearrange(
                        "b p c -> p b c"),
                )
                nc.scalar.dma_start(
                    x4[t + 1][:].rearrange("p (b c) -> p b c", b=B_LOC),
                    x_d[:, 128 * (t + 1):128 * (t + 2), :].rearrange(
                        "b p c -> p b c"),
                )
            wp1T = cpool.tile([128, NCH * C], BF, tag="wp1T")
            nc.scalar.dma_start(wp1T[:], wp1T_d[:])
            wprojT = cpool.tile([128, NCH * C], BF, tag="wprojT")
            nc.scalar.dma_start(wprojT[:], wprojT_d[:])
            wp2T = cpool.tile([128, NCH * C], BF, tag="wp2T")
            nc.sync.dma_start(wp2T[:], wp2T_d[:])
            xT = {}

            def load_xT(b, kcs=range(NCH), eng=None):
                for kc in kcs:
                    tt = xtpool.tile([128, N], BF, tag="xT",
                                     name=f"xT{b}_{kc}")
                    (eng or nc.sync).dma_start(
                        tt[:], x_d[b, :, 128 * kc:128 * (kc + 1)],
                        transpose=True,
                    )
                    xT[(b, kc)] = tt

            # b0-b2 channel-major via DMA transpose upfront (18 xt
            # bufs, no slot waits); b3 deferred into the main loop on
            # the scalar ring so its slot-recycle waits block nothing.
            for b in range(B_LOC - 1):
                load_xT(b)

            # ---------------- conv workspaces ----------------
            ws = []
            for i in range(NCH):
                a = wspool.tile([128, WS], BF, tag=f"ws{i}")
                nc.gpsimd.memset(a[:], 0.0)
                ws.append(a)

            # ---------------- pooling ----------------
            def emit_pool():
                NJ = B_LOC * C // 512  # 6 stripes of the (b,c) free dim
                pps = [(psA if j < 3 else psB).tile(
                    [KK, 512], F32, tag="psa" if j < 3 else "psb",
                    name=f"pp{j}") for j in range(NJ)]
                for t in range(NTC):
                    for j in range(NJ):
                        nc.tensor.matmul(
                            pps[j][:],
                            S_sb[:, KK * t:KK * (t + 1)],
                            x4[t][:, 512 * j:512 * (j + 1)],
                            start=(t == 0),
                            stop=(t == NTC - 1),
                        )
                xp4 = kgpool.tile([KK, B_LOC * C], BF, tag="xp4")
                for j in range(NJ):
                    nc.vector.tensor_copy(
                        xp4[:, 512 * j:512 * (j + 1)], pps[j][:])
                return xp4

            # ---------------- kernel generation (batched) ----------------
            ktAll = []

            def emit_kgen(xp4):
                xpT4 = []
                for i in range(NCH):
                    xpT4.append(ktpool.tile([128, B_LOC * KK], BF,
                                            tag="xpT", name=f"xpT{i}"))
                for b in range(B_LOC):
                    for i in range(NCH):
                        tp = psA.tile([128, KK], BF, tag="psa")
                        nc.tensor.transpose(
                            tp[:],
                            xp4[:, C * b + 128 * i:C * b + 128 * (i + 1)],
                            eye9b[:],
                        )
                        nc.vector.tensor_copy(
                            xpT4[i][:, KK * b:KK * (b + 1)], tp[:])

                tsil = kgpool.tile([B_LOC * KK, C], BF, tag="tsil")
                for h in range(2):
                    tp1 = psB.tile([B_LOC * KK, 384], F32, tag="psb",
                                   name=f"tp1{h}")
                    nc.tensor.matmul(
                        tp1[:], ones_sb[:1, :],
                        p1b_sb[:1, 384 * h:384 * (h + 1)],
                        start=True, stop=False,
                    )
                    for kc in range(NCH):
                        nc.tensor.matmul(
                            tp1[:], xpT4[kc][:],
                            wp1T[:, C * kc + 384 * h:C * kc + 384 * (h + 1)],
                            start=False, stop=(kc == NCH - 1),
                        )
                    sg = kgpool.tile([B_LOC * KK, 384], BF, tag=f"sg{h}")
                    nc.scalar.activation(
                        sg[:], tp1[:],
                        mybir.ActivationFunctionType.Sigmoid,
                    )
                    nc.vector.tensor_tensor(
                        tsil[:, 384 * h:384 * (h + 1)], tp1[:], sg[:],
                        mult,
                    )

                # kernel gen: one block-diagonal matmul per half covers
                # all 4 samples (kg4 = blockdiag(kgT x4))
                k4 = kgpool.tile([B_LOC * KK, C], F32, tag="k4")
                for h in range(2):
                    kp = psB.tile([B_LOC * KK, 384], F32, tag="psb")
                    nc.tensor.matmul(
                        kp[:], kg4_sb[:], tsil[:, 384 * h:384 * (h + 1)],
                        start=True, stop=True,
                    )
                    nc.scalar.activation(
                        k4[:, 384 * h:384 * (h + 1)], kp[:], IDENT,
                        bias=kgb4_sb[:],
                    )
                # ktAll[i]: [128, 36] f32, col 9b+j = tap TAP_ORDER[j] of
                # sample b (p36 = blockdiag(perm9 x4) permutes)
                for i in range(NCH):
                    tp = psB.tile([128, B_LOC * KK], F32, tag="psb")
                    nc.tensor.transpose(
                        tp[:], k4[:, 128 * i:128 * (i + 1)], p36_sb[:],
                    )
                    sb = ktpool.tile([128, B_LOC * KK], F32, tag="kT",
                                     name=f"ktA{i}")
                    nc.vector.tensor_copy(sb[:], tp[:])
                    ktAll.append(sb)

            # ---------------- main pipeline ----------------
            cv_tiles = {}

            def view(i, dy, dx, h=None):
                base = 36 * dy + dx
                rows = 32
                if h is not None:
                    base += 36 * 16 * h
                    rows = 16
                v = ws[i][:, base:base + 36 * rows]
                return v.rearrange("p (r e) -> p r e", e=36)[:, :, :32]

            def emit_p2(b, i):
                """p2 matmuls + PSUM->workspace evacuation."""
                ph = [psA.tile([128, 512], F32, tag="psa",
                               name=f"p2_{b}_{i}_{h}") for h in range(2)]
                for kc in range(NCH):
                    for h in range(2):
                        nc.tensor.matmul(
                            ph[h][:],
                            wp2T[:, C * kc + 128 * i:C * kc + 128 * (i + 1)],
                            xT[(b, kc)][:, 512 * h:512 * (h + 1)],
                            start=(kc == 0),
                            stop=(kc == NCH - 1),
                        )
                for h in range(2):
                    rb = 36 * (16 * h + 1) + 1
                    dst = ws[i][:, rb:rb + 36 * 16]
                    dst = dst.rearrange("p (r e) -> p r e", e=36)[:, :, :32]
                    src = ph[h][:].rearrange("p (r e) -> p r e", e=32)
                    nc.scalar.activation(dst, src, IDENT,
                                         bias=p2bT_sb[:, i:i + 1])

            def emit_convA(b, i):
                """dg build, DVE init + fused taps, ACT products."""
                ksc = ktAll[i]

                def kj(dy, dx):
                    c = KK * b + TAP_COL[(dy, dx)]
                    return ksc[:, c:c + 1]

                cv = cvpool.tile([128, N], BF, tag="cv", name=f"cv{b}_{i}")
                cv_tiles[(b, i)] = cv
                acc = cv[:].rearrange("p (r e) -> p r e", e=32)
                # dg for the PE taps (kT cols 9b+0..3)
                dg = dgpool.tile([128, NPE * 128], BF, tag="dg")
                nc.vector.tensor_tensor(
                    dg[:].rearrange("p (j f) -> p j f", f=128),
                    eye3x[:].rearrange("p (j f) -> p j f", f=128),
                    ksc[:, KK * b:KK * b + NPE].broadcast_to(
                        (128, NPE, 128)),
                    mult,
                )
                # accumulator init + fused taps
                nc.vector.tensor_scalar(
                    acc, view(i, *TS_INIT), kj(*TS_INIT), None, mult)
                for dy, dx in STT_TAPS:
                    nc.vector.scalar_tensor_tensor(
                        acc, view(i, dy, dx), kj(dy, dx), acc, mult, add)
                # ACT products
                tmps = []
                for dy, dx in ACT_TAPS:
                    tm = tmppool.tile([128, N], BF, tag="tmp")
                    nc.scalar.activation(
                        tm[:].rearrange("p (r e) -> p r e", e=32),
                        view(i, dy, dx), IDENT, scale=kj(dy, dx))
                    tmps.append(tm)
                return (b, i, cv, dg, tmps)

            def emit_convB(carry):
                """PE conv taps, partial evac, folds."""
                b, i, cv, dg, tmps = carry
                pc = [psA.tile([128, 512], F32, tag="psa",
                               name=f"pc_{b}_{i}_{h}") for h in range(2)]
                for j, (dy, dx) in enumerate(PE_TAPS):
                    for h in range(2):
                        nc.tensor.matmul(
                            pc[h][:],
                            dg[:, 128 * j:128 * (j + 1)],
                            view(i, dy, dx, h),
                            start=(j == 0),
                            stop=(j == len(PE_TAPS) - 1),
                        )
                tpe = tmppool.tile([128, N], BF, tag="tmp")
                for h in range(2):
                    nc.scalar.activation(
                        tpe[:, 512 * h:512 * (h + 1)], pc[h][:], IDENT)
                for tm in tmps:
                    nc.vector.tensor_tensor(cv[:], tm[:], cv[:], add)
                nc.vector.tensor_tensor(cv[:], tpe[:], cv[:], add)

            def proj_t(b, t):
                po = [psB.tile([128, 384], F32, tag="psb",
                               name=f"po{b}_{t}_{h}") for h in range(2)]
                for kc in range(NCH):
                    for h in range(2):
                        nc.tensor.matmul(
                            po[h][:],
                            cv_tiles[(b, kc)][:, 128 * t:128 * (t + 1)],
                            wprojT[:, C * kc + 384 * h:
                                   C * kc + 384 * (h + 1)],
                            start=(kc == 0),
                            stop=(kc == NCH - 1),
                        )
                osb = osbpool.tile([128, C], F32, tag="osb")
                nc.vector.tensor_copy(osb[:, 0:384], po[0][:])
                nc.scalar.activation(osb[:, 384:768], po[1][:], IDENT)
                nc.sync.dma_start(
                    out_d[b, 128 * t:128 * (t + 1), :], osb[:],
                )

            # proj t-blocks of the previous sample interleave with this
            # sample's chunks so their evacuations never queue behind a
            # whole sample of conv work.
            TSLOT = [(0,), (1,), (2,), (3,), (4, 5), (6, 7)]

            # startup: pool fills the PE while x streams in; kgen runs
            # as soon as pooling lands (it gates every conv chunk);
            # then b0's p2; then a uniform software pipeline (p2 two
            # chunks ahead, conv one behind).
            xp4 = emit_pool()
            for i in range(3):
                emit_p2(0, i)
            emit_kgen(xp4)
            for i in range(3, NCH):
                emit_p2(0, i)

            chunks = [(b, i) for b in range(B_LOC) for i in range(NCH)]
            NCK = len(chunks)
            carry = None
            for g in range(NCK):
                if 5 <= g <= 10:
                    load_xT(3, [g - 5], eng=nc.scalar)
                if 6 <= g + 2 < NCK:
                    emit_p2(*chunks[g + 2])
                nxt = emit_convA(*chunks[g])
                if carry is not None:
                    emit_convB(carry)
                carry = nxt
                b, i = chunks[g]
                if b >= 1:
                    for t in TSLOT[i]:
                        proj_t(b - 1, t)
            emit_convB(carry)
            for t in range(NTC):
                proj_t(B_LOC - 1, t)

    nc.finalize()
    return nc


def _prepare_weights(inputs):
    bf = ml_dtypes.bfloat16

    def packT(w):
        # [C_out, C_in] -> transposed [C_in, C_out], chunked over the
        # contraction dim: chunk kc (rows 128*kc..) at cols C*kc..C*(kc+1)
        wT = np.ascontiguousarray(np.asarray(w, np.float32).T)
        return np.ascontiguousarray(
            wT.reshape(NCH, 128, C).transpose(1, 0, 2).reshape(128, NCH * C)
        ).astype(bf)

    p1_b = np.asarray(inputs["p1_b"], np.float32)
    kg_w = np.asarray(inputs["kg_w"], np.float32)
    kg_b = np.asarray(inputs["kg_b"], np.float32)
    p2_b = np.asarray(inputs["p2_b"], np.float32)
    beta = np.asarray(inputs["beta"], np.float32)

    factor = 1.0 / (1.0 + np.exp(-beta))
    assert np.allclose(factor, factor[0], atol=1e-6), (
        "non-uniform sigmoid(beta) not supported by the host fold"
    )
    A = np.eye(KK, dtype=np.float32) - float(factor[0]) / KK
    kg_w_eff = (A @ kg_w).astype(np.float32)
    kg_b_eff = (A @ kg_b).astype(np.float32)

    S = _segment_matrix()  # [N, KK]
    S_packed = np.ascontiguousarray(
        S.reshape(NTC, 128, KK).transpose(1, 0, 2).reshape(128, NTC * KK)
    ).astype(bf)

    npe = len(PE_TAPS)
    eye3x = np.zeros((128, npe * 128), np.float32)
    for j in range(npe):
        eye3x[:, 128 * j:128 * (j + 1)] = np.eye(128)

    # permutation: transpose output column TAP_COL[t] takes k row 3dy+dx
    perm9 = np.zeros((KK, KK), np.float32)
    for (dy, dx), col in TAP_COL.items():
        perm9[3 * dy + dx, col] = 1.0
    p36 = np.zeros((4 * KK, 4 * KK), np.float32)
    kg4 = np.zeros((4 * KK, 4 * KK), np.float32)
    kgT = kg_w_eff.T
    for b in range(B_LOC):
        p36[KK * b:KK * (b + 1), KK * b:KK * (b + 1)] = perm9
        kg4[KK * b:KK * (b + 1), KK * b:KK * (b + 1)] = kgT

    return {
        "wp2T": packT(inputs["p2_w"]),
        "wp1T": packT(inputs["p1_w"]),
        "wprojT": packT(inputs["proj_w"]),
        "S": S_packed,
        "kg4": kg4.astype(bf),
        "p1b": p1_b.reshape(1, C).astype(bf),
        "p2bT": np.ascontiguousarray(p2_b.reshape(NCH, 128).T),
        "kgb4": np.ascontiguousarray(np.tile(kg_b_eff, 4).reshape(4 * KK, 1)),
        "ones": np.ones((1, 4 * KK), bf),
        "eye9b": np.eye(KK, dtype=np.float32).astype(bf),
        "p36": p36,
        "eye3x": eye3x.astype(bf),
    }


def kernel(**inputs):
    global LAST_RESULTS
    if "nc" not in _CACHE:
        _CACHE["nc"] = build_program()
    nc = _CACHE["nc"]

    x = np.asarray(inputs["x"], np.float32)
    weights = _prepare_weights(inputs)
    xbf = x.astype(ml_dtypes.bfloat16)

    in_maps = []
    for c in range(N_CORES):
        m = dict(weights)
        m["xbf"] = np.ascontiguousarray(xbf[B_LOC * c:B_LOC * (c + 1)])
        in_maps.append(m)

    res = run_bass_kernel_spmd(nc, in_maps, list(range(N_CORES)))
    LAST_RESULTS = res
    out = np.concatenate([res.results[c]["out"] for c in range(N_CORES)],
                         axis=0)
    out = out.astype(np.float32)
    out += np.asarray(inputs["proj_b"], np.float32)[None, None, :]
    return np.ascontiguousarray(out)

